# revision 18
# baseline (speedup 1.0000x reference)
"""Trainium2 Bass kernel for nn_PolyHarmonicOscillator.

Problem (hardcoded shapes): batch=2, poly=8, harm=64, frames=500,
FRAME_LENGTH=64 -> audio = 32000 samples, SR=16000.

  f0_bank = f0 * (h+1)            (2, 512, 500)
  c_bank  = c * (f0_bank < SR/2)
  phase   = cumsum(upsample64(f0_bank)/SR) + random_phases
  out     = sum_osc sin(2*pi*phase) * 0.04 * upsample64(c_bank) * upsample64(v)

Sharding: core k takes poly-voice k: 128 rows = 2 batches x 64 harmonics on
SBUF partitions.  Host sums the 8 per-core partial audio outputs.

Math decomposition (validated vs fp64 reference, ~5e-4 absmax vs the jax fp32
reference's own 7.7e-3 fp32 envelope):
 * Per output sample j = 64*t + s the linear upsample is a 2-tap interp with
   per-phase weights that are multiples of 1/128.  The phase cumsum splits into
   a per-frame carry C[t] (500-long scan of closed-form block sums
   S[t] = 8u[t-1] + 48u[t] + 8u[t+1], u = f0_bank/SR, with S reduced mod 1
   BEFORE the scan so the fp32 accumulator stays < 500) plus a closed-form
   3-tap within-block term A(s)u[t-1]+B(s)u[t]+D(s)u[t+1].
 * amp = up(c_bank)*up(v) decomposes over q0[t] = G*c_bank[t]*v[t] and
   cr[t] = G*(c_bank[t-1]v[t] + c_bank[t]v[t-1]) with per-phase 3-tap weights
   ((1-w)^2, w(1-w), w^2);  G = -0.04 (sign folds the sin( . - pi) flip).
 * Both 3-tap forms become TensorE matmuls with FIXED [k,128] weights over a
   t-major staged rhs; output layout [(dt,s)=128 partitions, (pair, b, osc)].
 * Edge clamping of the torch-style upsample is handled exactly by halos
   u[-1]=u[0], u[500]=u[499], q0 same, cr[0]=2q0[0], cr[500]=2q0[499].

This file is self-contained (no reference.py / spec.json imports).
"""

import os
import numpy as np

SR = 16000.0
GAIN = -0.04          # -(0.04): compensates sin(2 pi x - pi) = -sin(2 pi x)
BATCH, POLY, HARM, FRAMES = 2, 8, 64, 500
NCORES = 8
PAIRS = FRAMES // 2   # 250 pairs of frames; 128 output samples per pair
CHUNK_F = 100         # frames per staging chunk
CHUNK_P = CHUNK_F // 2
NCHUNK = FRAMES // CHUNK_F
TILE_PAIRS = 4        # pairs per big-matmul tile (n = 512 columns)


# ---------------------------------------------------------------------------
# Host-side constant tables
# ---------------------------------------------------------------------------
def _coef_tables():
    s = np.arange(64)
    wL = (s + 0.5) / 64 + 0.5     # pair (t-1, t), s <= 31
    wR = (s + 0.5) / 64 - 0.5     # pair (t, t+1), s >= 32

    A = np.zeros(64); B = np.zeros(64); D = np.zeros(64)
    accA = accB = 0.0
    for i in range(32):
        accA += 1 - wL[i]; accB += wL[i]
        A[i] = accA; B[i] = accB
    accD = 0.0
    for i in range(32, 64):
        accB += 1 - wR[i]; accD += wR[i]
        A[i] = accA; B[i] = accB; D[i] = accD

    P_m1 = np.zeros(64); P_0 = np.zeros(64); P_p1 = np.zeros(64)
    X_0 = np.zeros(64); X_p1 = np.zeros(64)
    for i in range(32):
        w = wL[i]
        P_m1[i] = (1 - w) ** 2; P_0[i] = w * w; X_0[i] = w * (1 - w)
    for i in range(32, 64):
        w = wR[i]
        P_0[i] = (1 - w) ** 2; P_p1[i] = w * w; X_p1[i] = w * (1 - w)
    return A, B, D, P_m1, P_0, P_p1, X_0, X_p1


def _build_consts():
    A, B, D, P_m1, P_0, P_p1, X_0, X_p1 = _coef_tables()

    def tap_pattern(Ax, Bx, Dx):
        """[4, 128] weights for taps {u[2P-1], u[2P], u[2P+1], u[2P+2]}."""
        w = np.zeros((4, 128), np.float64)
        for dt in range(2):
            for s in range(64):
                p = dt * 64 + s
                if dt == 0:
                    w[0, p] = Ax[s]; w[1, p] = Bx[s]; w[2, p] = Dx[s]
                else:
                    w[1, p] = Ax[s]; w[2, p] = Bx[s]; w[3, p] = Dx[s]
        return w

    # fp16 hi/lo split of the phase coefficients
    A1 = A.astype(np.float16).astype(np.float64)
    B1 = B.astype(np.float16).astype(np.float64)
    D1 = D.astype(np.float16).astype(np.float64)
    A2, B2, D2 = A - A1, B - B1, D - D1

    ind = np.zeros((4, 128), np.float64)   # pc1/pc2 indicator rows
    for dt in range(2):
        for s in range(64):
            p = dt * 64 + s
            ind[dt, p] = 1.0
            ind[2 + dt, p] = 1.0

    # mm1: rows {pc1 e/o, pc2 e/o, u1 x4} ; mm2: rows {u1 x4, u2 x4}
    phw1 = np.concatenate([ind, tap_pattern(A1, B1, D1)], axis=0)   # [8,128]
    phw2 = np.concatenate([tap_pattern(A2, B2, D2),
                           tap_pattern(A1, B1, D1)], axis=0)        # [8,128]

    ampw = np.zeros((7, 128), np.float64)
    for dt in range(2):
        for s in range(64):
            p = dt * 64 + s
            if dt == 0:
                ampw[0, p] = P_m1[s]; ampw[1, p] = P_0[s]; ampw[2, p] = P_p1[s]
                ampw[4, p] = X_0[s]; ampw[5, p] = X_p1[s]
            else:
                ampw[1, p] = P_m1[s]; ampw[2, p] = P_0[s]; ampw[3, p] = P_p1[s]
                ampw[5, p] = X_0[s]; ampw[6, p] = X_p1[s]

    # per-partition multipliers (h+1)/SR for u = f0_rep * pmk
    pmk = np.zeros((128, 1), np.float64)
    for b in range(2):
        for h in range(64):
            pmk[b * 64 + h, 0] = (h + 1) / SR

    ident = np.eye(128, dtype=np.float16)
    return {
        "phw1": phw1.astype(np.float16),
        "phw2": phw2.astype(np.float16),
        "ampw": ampw.astype(np.float16),
        "pmk": pmk.astype(np.float32),
        "ident": ident,
    }


# ---------------------------------------------------------------------------
# Bass module
# ---------------------------------------------------------------------------
def build_bass():
    from contextlib import ExitStack
    import concourse.bacc as bacc
    import concourse.tile as tile
    import concourse.mybir as mybir

    f32 = mybir.dt.float32
    f16 = mybir.dt.float16
    ALU = mybir.AluOpType
    ACTF = mybir.ActivationFunctionType

    nc = bacc.Bacc("TRN2", debug=False, target_bir_lowering=False,
                   num_devices=NCORES, enable_partition_id=False)

    d_f0 = nc.dram_tensor("f0k", [128, FRAMES], f32, kind="ExternalInput")
    d_c = nc.dram_tensor("ck", [128, FRAMES], f32, kind="ExternalInput")
    d_v = nc.dram_tensor("vk", [128, FRAMES], f32, kind="ExternalInput")
    d_rp = nc.dram_tensor("rpk", [128, 1], f32, kind="ExternalInput")
    d_phw1 = nc.dram_tensor("phw1", [8, 128], f16, kind="ExternalInput")
    d_phw2 = nc.dram_tensor("phw2", [8, 128], f16, kind="ExternalInput")
    d_ampw = nc.dram_tensor("ampw", [7, 128], f16, kind="ExternalInput")
    d_pm = nc.dram_tensor("pmk", [128, 1], f32, kind="ExternalInput")
    d_id = nc.dram_tensor("ident", [128, 128], f16, kind="ExternalInput")
    d_out = nc.dram_tensor("audio", [2, 32000], f32, kind="ExternalOutput")

    HF = FRAMES + 2  # 502: halo col i <-> frame i-1

    with tile.TileContext(nc) as tc, ExitStack() as ctx:
        pers = ctx.enter_context(tc.tile_pool(name="pers", bufs=1))
        tmp = ctx.enter_context(tc.tile_pool(name="tmp", bufs=2))
        stage_p = ctx.enter_context(tc.tile_pool(name="stage_p", bufs=2))
        stage_a = ctx.enter_context(tc.tile_pool(name="stage_a", bufs=2))
        tr_sb = ctx.enter_context(tc.tile_pool(name="tr_sb", bufs=3))
        big = ctx.enter_context(tc.tile_pool(name="big", bufs=3))
        ps_small = ctx.enter_context(
            tc.tile_pool(name="ps_small", bufs=2, space="PSUM"))
        ps_ph = ctx.enter_context(
            tc.tile_pool(name="ps_ph", bufs=2, space="PSUM"))
        ps_amp = ctx.enter_context(
            tc.tile_pool(name="ps_amp", bufs=2, space="PSUM"))

        # ---- load inputs -------------------------------------------------
        f0_sb = pers.tile([128, FRAMES], f32)
        nc.sync.dma_start(f0_sb[:], d_f0[:])
        c_sb = pers.tile([128, FRAMES], f32)
        nc.sync.dma_start(c_sb[:], d_c[:])
        v_sb = pers.tile([128, FRAMES], f32)
        nc.sync.dma_start(v_sb[:], d_v[:])
        rp_sb = pers.tile([128, 1], f32)
        nc.sync.dma_start(rp_sb[:], d_rp[:])
        phw1_sb = pers.tile([8, 128], f16)
        nc.sync.dma_start(phw1_sb[:], d_phw1[:])
        phw2_sb = pers.tile([8, 128], f16)
        nc.sync.dma_start(phw2_sb[:], d_phw2[:])
        ampw_sb = pers.tile([7, 128], f16)
        nc.sync.dma_start(ampw_sb[:], d_ampw[:])
        pm_sb = pers.tile([128, 1], f32)
        nc.sync.dma_start(pm_sb[:], d_pm[:])
        id_sb = pers.tile([128, 128], f16)
        nc.sync.dma_start(id_sb[:], d_id[:])

        # ---- small per-frame compute (r-layout) ---------------------------
        # u = f0_rep * (h+1)/SR  with halo cols
        u_sb = pers.tile([128, HF], f32)
        nc.vector.tensor_scalar(
            u_sb[:, 1:501], f0_sb[:], pm_sb[:], None, ALU.mult)
        nc.vector.tensor_copy(u_sb[:, 0:1], u_sb[:, 1:2])
        nc.vector.tensor_copy(u_sb[:, 501:502], u_sb[:, 500:501])

        # vG = G * v
        vg_sb = pers.tile([128, FRAMES], f32)
        nc.vector.tensor_scalar(vg_sb[:], v_sb[:], float(GAIN), None, ALU.mult)

        # cm = (u < 0.5) * c      (anti-alias mask)
        cm_sb = pers.tile([128, FRAMES], f32)
        nc.vector.scalar_tensor_tensor(
            cm_sb[:], u_sb[:, 1:501], 0.5, c_sb[:], ALU.is_lt, ALU.mult)

        # qc: packed f16 [q0 | cr], each with halo layout
        qc_sb = pers.tile([128, 2 * HF], f16)
        q0h = qc_sb[:, 0:HF]
        crh = qc_sb[:, HF:2 * HF]
        nc.vector.tensor_tensor(q0h[:, 1:501], cm_sb[:], vg_sb[:], ALU.mult)
        nc.vector.tensor_copy(q0h[:, 0:1], q0h[:, 1:2])
        nc.vector.tensor_copy(q0h[:, 501:502], q0h[:, 500:501])

        x1_sb = tmp.tile([128, FRAMES - 1], f32)
        nc.vector.tensor_tensor(
            x1_sb[:], cm_sb[:, 0:499], vg_sb[:, 1:500], ALU.mult)
        x2_sb = tmp.tile([128, FRAMES - 1], f32)
        nc.vector.tensor_tensor(
            x2_sb[:], cm_sb[:, 1:500], vg_sb[:, 0:499], ALU.mult)
        nc.vector.tensor_tensor(crh[:, 2:501], x1_sb[:], x2_sb[:], ALU.add)
        nc.vector.tensor_scalar(crh[:, 1:2], q0h[:, 1:2], 2.0, None, ALU.mult)
        nc.vector.tensor_scalar(
            crh[:, 501:502], q0h[:, 500:501], 2.0, None, ALU.mult)
        nc.vector.memset(crh[:, 0:1], 0.0)

        # S[t] = (6*u[t] + (u[t-1]+u[t+1])) * 8, then centered residue
        # S - round(S) via the fp32 magic-number trick (phase only matters
        # mod 1).  1.5*2^23 keeps x+MAGIC in [2^23, 2^24) for x in +-2^22,
        # where the fp32 ulp is exactly 1.0 (plain 2^23 breaks for x < 0).
        TWO23 = 12582912.0
        sm_sb = tmp.tile([128, FRAMES], f32)
        t8_sb = tmp.tile([128, FRAMES], f32)
        rn_sb = tmp.tile([128, FRAMES], f32)
        nc.vector.tensor_tensor(
            t8_sb[:], u_sb[:, 0:500], u_sb[:, 2:502], ALU.add)
        nc.vector.scalar_tensor_tensor(
            t8_sb[:], u_sb[:, 1:501], 6.0, t8_sb[:], ALU.mult, ALU.add)
        nc.vector.tensor_scalar(
            rn_sb[:], t8_sb[:], 8.0, TWO23, ALU.mult, ALU.add)
        nc.vector.tensor_scalar(
            rn_sb[:], rn_sb[:], TWO23, None, ALU.subtract)
        nc.vector.scalar_tensor_tensor(
            sm_sb[:], t8_sb[:], 8.0, rn_sb[:], ALU.mult, ALU.subtract)

        # exclusive scan: C[0]=0, C[t] = sum_{j<t} S[j]
        zer_sb = tmp.tile([128, FRAMES], f32)
        nc.vector.memset(zer_sb[:], 0.0)
        c_scan = tmp.tile([128, FRAMES], f32)
        nc.vector.memset(c_scan[:, 0:1], 0.0)
        nc.vector.tensor_tensor_scan(
            c_scan[:, 1:500], sm_sb[:, 0:499], zer_sb[:, 0:499],
            0.0, ALU.add, ALU.add)

        # pc = (C - round(C)) + rp   (centered residue)
        pc_sb = tmp.tile([128, HF], f32)
        rnc_sb = tmp.tile([128, FRAMES], f32, tag="rnc")
        nc.vector.tensor_scalar(
            rnc_sb[:], c_scan[:], TWO23, TWO23, ALU.add, ALU.subtract)
        nc.vector.scalar_tensor_tensor(
            pc_sb[:, 1:501], c_scan[:], rp_sb[:], rnc_sb[:],
            ALU.add, ALU.subtract)
        nc.vector.memset(pc_sb[:, 0:1], 0.0)
        nc.vector.memset(pc_sb[:, 501:502], 0.0)

        # ---- fp16 hi/lo splits, packed pairs for transposes ---------------
        # pc12 = [pc1 | pc2], u12 = [u1 | u2]  (f16, halo layout each)
        pc12_sb = pers.tile([128, 2 * HF], f16)
        u12_sb = pers.tile([128, 2 * HF], f16)
        lo32 = tmp.tile([128, HF], f32, tag="lo32")
        nc.vector.tensor_copy(pc12_sb[:, 0:HF], pc_sb[:])
        nc.vector.tensor_copy(lo32[:], pc12_sb[:, 0:HF])
        nc.vector.tensor_tensor(
            pc12_sb[:, HF:2 * HF], pc_sb[:], lo32[:], ALU.subtract)
        lo32b = tmp.tile([128, HF], f32, tag="lo32")
        nc.vector.tensor_copy(u12_sb[:, 0:HF], u_sb[:])
        nc.vector.tensor_copy(lo32b[:], u12_sb[:, 0:HF])
        nc.vector.tensor_tensor(
            u12_sb[:, HF:2 * HF], u_sb[:], lo32b[:], ALU.subtract)

        # bias constant 2^23 for the big-phase round trick
        two23_sb = pers.tile([128, 1], f32)
        nc.vector.memset(two23_sb[:], TWO23)

        # ---- audio accumulator: [128=(dt,s), 500 = b*250 + P] -------------
        audio_sb = pers.tile([128, 500], f32)
        audio_v = audio_sb[:].rearrange("p (b t) -> p t b", b=2)  # [128,250,2]

        # ---- per-chunk staging + big matmuls ------------------------------
        # parity windows: par=0 (odd frames 2p0-1+2i), par=1 (even 2p0+2i)
        WCNT = CHUNK_P + 1  # 51
        for g in range(NCHUNK):
            p0 = g * CHUNK_P
            c0 = 2 * p0                  # col of frame 2p0-1

            # staging rows (all f16):
            #  st_ph: 0-1 pc1 e/o, 2-3 pc2 e/o, 4-7 u1 taps, 8-11 u2 taps
            #  st_amp: 0-3 q0 taps, 4-6 cr taps
            st_ph1 = stage_p.tile([8, CHUNK_P * 128], f16, tag="st1")
            st_ph2 = stage_p.tile([8, CHUNK_P * 128], f16, tag="st2")
            st_amp = stage_a.tile([7, CHUNK_P * 128], f16)

            def stage_pair(packed_sb, dsts, tag):
                """Transpose each half of a packed [A|B] f16 tensor's parity
                windows ([128, 51] strided views, one free dim) and copy
                contiguous partition slices into staging rows.
                dsts: list of (dst_tile, rows); rows entries are
                (row, start, parity, half) with half 0=A, 1=B."""
                evs = {}
                for par in range(2):
                    for half in range(2):
                        if not any(r[2] == par and r[3] == half
                                   for _, rows in dsts for r in rows):
                            continue
                        ps_t = ps_small.tile([WCNT, 128], f16, tag="ps_tr")
                        b0 = half * HF + c0 + par
                        win = packed_sb[:, b0:b0 + 2 * WCNT - 1:2]
                        nc.tensor.transpose(ps_t[:], win, id_sb[:])
                        ev = tr_sb.tile([WCNT, 128], f16,
                                        tag=f"ev{tag}{par}{half}")
                        nc.any.tensor_copy(ev[:], ps_t[:])
                        evs[(par, half)] = ev
                for dst_tile, rows in dsts:
                    for row, start, par, half in rows:
                        nc.gpsimd.dma_start(
                            dst_tile[row:row + 1, :],
                            evs[(par, half)][start:start + CHUNK_P, :])

            stage_pair(pc12_sb,
                       [(st_ph1, [(0, 0, 1, 0), (1, 1, 0, 0),
                                  (2, 0, 1, 1), (3, 1, 0, 1)])], "pc")
            # u1 taps go to st_ph1 rows 4-7 (for phw1) AND st_ph2 rows 0-3
            # (for the A2-correction matmul); u2 taps to st_ph2 rows 4-7.
            stage_pair(u12_sb,
                       [(st_ph1, [(4, 0, 0, 0), (5, 0, 1, 0), (6, 1, 0, 0),
                                  (7, 1, 1, 0)]),
                        (st_ph2, [(0, 0, 0, 0), (1, 0, 1, 0), (2, 1, 0, 0),
                                  (3, 1, 1, 0),
                                  (4, 0, 0, 1), (5, 0, 1, 1), (6, 1, 0, 1),
                                  (7, 1, 1, 1)])], "u")
            stage_pair(qc_sb,
                       [(st_amp, [(0, 0, 0, 0), (1, 0, 1, 0), (2, 1, 0, 0),
                                  (3, 1, 1, 0),
                                  (4, 0, 1, 1), (5, 1, 0, 1),
                                  (6, 1, 1, 1)])], "qc")

            ntile = (CHUNK_P + TILE_PAIRS - 1) // TILE_PAIRS
            for it in range(ntile):
                pt = it * TILE_PAIRS
                npair = min(TILE_PAIRS, CHUNK_P - pt)
                n = npair * 128
                off = pt * 128

                psph = ps_ph.tile([128, TILE_PAIRS * 128], f32)
                nc.tensor.matmul(psph[:, :n], phw1_sb[:],
                                 st_ph1[:, off:off + n],
                                 start=True, stop=False)
                nc.tensor.matmul(psph[:, :n], phw2_sb[:],
                                 st_ph2[:, off:off + n],
                                 start=False, stop=True)
                psam = ps_amp.tile([128, TILE_PAIRS * 128], f32)
                nc.tensor.matmul(psam[:, :n], ampw_sb[:],
                                 st_amp[:, off:off + n], start=True, stop=True)

                # centered residue: rn - x in [-0.5, 0.5]; sin(2pi(rn-x))
                # = -sin(2pi x), sign folded into GAIN.
                sh_t = big.tile([128, TILE_PAIRS * 128], f32, tag="sh")
                nc.scalar.activation(
                    sh_t[:, :n], psph[:, :n], ACTF.Identity,
                    bias=two23_sb[:])
                pm_t = big.tile([128, TILE_PAIRS * 128], f32, tag="pm")
                nc.vector.scalar_tensor_tensor(
                    pm_t[:, :n], sh_t[:, :n], TWO23, psph[:, :n],
                    ALU.subtract, ALU.subtract)
                sin_t = big.tile([128, TILE_PAIRS * 128], f16, tag="sin")
                nc.scalar.activation(
                    sin_t[:, :n], pm_t[:, :n], ACTF.Sin,
                    scale=float(2 * np.pi))
                amp_t = big.tile([128, TILE_PAIRS * 128], f16, tag="amp")
                nc.scalar.copy(amp_t[:, :n], psam[:, :n])
                prod_t = big.tile([128, TILE_PAIRS * 128], f16, tag="prod")
                nc.vector.tensor_tensor(
                    prod_t[:, :n], sin_t[:, :n], amp_t[:, :n], ALU.mult)

                pv = prod_t[:, :n].rearrange("p (g o) -> p g o", o=64)
                nc.vector.tensor_reduce(
                    audio_v[:, p0 + pt:p0 + pt + npair, :], pv,
                    mybir.AxisListType.X, ALU.add)

        # ---- output: f16 transpose [128, 125]-slices and DMA out ----------
        aud16 = pers.tile([128, 500], f16)
        nc.vector.tensor_copy(aud16[:], audio_sb[:])
        d_out_v = d_out[:].rearrange("b (t x) -> b t x", x=128)  # [2,250,128]
        for b in range(2):
            for hlf in range(2):
                col = b * 250 + hlf * 125
                ps_o = ps_small.tile([125, 128], f16, tag="ps_tr")
                nc.tensor.transpose(
                    ps_o[:], aud16[:, col:col + 125], id_sb[:])
                ot = tr_sb.tile([125, 128], f32, tag="out")
                nc.scalar.copy(ot[:], ps_o[:])
                nc.sync.dma_start(
                    d_out_v[b, hlf * 125:hlf * 125 + 125, :], ot[:])

    nc.compile()
    return nc


# ---------------------------------------------------------------------------
# Host driver
# ---------------------------------------------------------------------------
_NC = None


def _get_nc():
    global _NC
    if _NC is None:
        _NC = build_bass()
    return _NC


def _make_in_maps(f0, c, v, random_phases):
    consts = _build_consts()
    f0 = np.ascontiguousarray(np.asarray(f0, np.float32))
    c = np.ascontiguousarray(np.asarray(c, np.float32))
    v = np.ascontiguousarray(np.asarray(v, np.float32))
    rp = np.ascontiguousarray(np.asarray(random_phases, np.float32))
    in_maps = []
    for k in range(NCORES):
        # replicate f0/v across the 64 harmonic rows (layout only)
        f0r = np.repeat(f0[:, k, :][:, None, :], HARM, axis=1).reshape(128, FRAMES)
        vr = np.repeat(v[:, k, :][:, None, :], HARM, axis=1).reshape(128, FRAMES)
        m = {
            "f0k": np.ascontiguousarray(f0r),
            "ck": np.ascontiguousarray(c[:, k, :, :].reshape(128, FRAMES)),
            "vk": np.ascontiguousarray(vr),
            "rpk": np.ascontiguousarray(
                rp[:, 64 * k:64 * (k + 1), 0].reshape(128, 1)),
        }
        m.update(consts)
        in_maps.append(m)
    return in_maps


def _ensure_ntff_hook():
    """Register the axon NTFF profile hook if the image's antenv lacks it."""
    import sys as _sys
    import types as _types
    try:
        from antenv import axon_hooks  # noqa: F401
        return
    except ImportError:
        pass
    try:
        from trn_agent_boot.trn_boot import _ntff_profile_via_ctypes
        hook = _ntff_profile_via_ctypes("/opt/axon/libaxon_pjrt.so")
        mod = _types.ModuleType("antenv.axon_hooks")
        mod.get_axon_ntff_profile_hook = lambda: hook
        mod.set_axon_ntff_profile_hook = lambda h: None
        _sys.modules["antenv.axon_hooks"] = mod
        import antenv
        antenv.axon_hooks = mod
    except Exception:
        pass


def run(f0, c, v, a=None, random_phases=None, trace=False):
    """Run the SPMD kernel; returns (audio, exec_time_ns_or_None)."""
    from concourse.bass_utils import run_bass_kernel_spmd
    if trace:
        _ensure_ntff_hook()
    nc = _get_nc()
    in_maps = _make_in_maps(f0, c, v, random_phases)
    res = run_bass_kernel_spmd(
        nc, in_maps, core_ids=list(range(NCORES)), trace=trace)
    out = np.zeros((2, 32000), np.float32)
    for r in res.results:
        out += r["audio"]
    return out, res.exec_time_ns


def kernel(f0, c, v, a, random_phases):
    out, _ = run(f0, c, v, a, random_phases,
                 trace=bool(int(os.environ.get("KERNEL_TRACE", "0"))))
    return out
